# revision 47
# baseline (speedup 1.0000x reference)
"""Trainium2 Bass kernel for nn_Conv2dGeneral (capsule-style 4x4-pose conv).

Math (from the reference):
  out[b,o,X,Y,u,w] = sum_{cin,kx,ky,v} Wm[(cin,kx,ky),o,u,v] * x[b,cin,2X+kx,2Y+ky,4v+w] + bias[o]

Mapped to the PE array as a single 1152-deep contraction:
  K = (cin, v)  x  9 accumulation chunks over (kx, ky)   [9 x 128]
  M = (o, u)                                              [128 PSUM partitions]
  N = (X, Y, w)                                           [676 per batch image]

Data-parallel across 8 NeuronCores on the batch dim (8 images per core).

Timeline engineering (v5 — what the traces showed):
  - exec_time == last-output-byte time + ~1.5us: a fixed ~7.9us runtime
    epilogue (256 semaphore resets + final barrier) minus a fixed
    ~6.4us preamble. Only the first-data -> last-output span matters.
  - All inputs ride the sync HW queue (it starves the other queues
    while it has backlog, which protects the input stream). Each DMA
    engine moves ~22GB/s; 16 engines saturate the ~358GB/s core cap
    only with dense big descriptors, so w is PACKED WITH batch 0's head
    rows in one DRAM tensor and one descriptor -- the stream opens
    dense, and the first matmul gate (w AND b0 head) clears ~3.5us
    sooner than separate small descriptors.
  - Batches 1..4 land as head/tail halves (finer completion gating
    where the PE chases the stream); b5..b7 as single descriptors
    (fewer descriptors = less DGE work = smaller straggler-engine lag
    on every completion semaphore).
  - Warm-up matmuls on garbage SBUF (ot staging tile) from preamble
    exit: the PE HAM clock gate holds the array at half duty until
    ~6us of sustained activity, so the warm-ups both hide the DMA
    latency and buy the clock ramp. Real groups start at half clock
    as soon as data lands; the ramp latches mid-flight.
  - Output fp16; ACT (bias add) per PSUM group on scalar; one
    whole-image store per batch 0..6 on the scalar HW queue (starved
    behind inputs -- harmless). Batch 7 is split (7,5,1)xWOUT with its
    first two stores issued by gpsimd in parallel with scalar's ACTs
    and a final 52-column store by scalar, so the post-matmul tail is
    ~1.3us instead of ~3.
  - Row 27 / col 27 of each 28x28 image are never read by the stride-2
    K=3 window and are not transferred (host repack to 27x27).
  - A dummy activation before the output loop hoists the 1.3us
    ACT_TABLE_LOAD into the DMA shadow.
"""

import numpy as np

B, CIN, COUT = 64, 32, 32
KK, STRIDE = 3, 2
WIN, HH = 28, 16
H = 4
WOUT = (WIN - KK) // STRIDE + 1  # 13
NCORES = 8
BPC = B // NCORES                # batches per core
RDIM = 27                        # rows/cols actually read (row/col 27 dropped)
ROWE = RDIM * H                  # 108 elements per image row per partition
RC = RDIM * ROWE                 # 2916 free elements per (cin,v) partition
NOUT = WOUT * WOUT * H           # 676 outputs per (o,u) partition per image
XGRP = WOUT * H                  # 52 output elements per X row
WCOLS = 9 * 128                  # 1152 weight columns per partition
HCUT = 15 * ROWE                 # image head rows [0,15) / tail rows [15,27)
NSPLIT = 5                       # batches 1..NSPLIT-1 are head/tail split

# PSUM groups (batch, X0, nX). Batch 0 in three X splits (the first gate is
# just w + 9 image rows). Batches 6 and 7 are interleaved and batch 7 ends
# in a tiny 52-col group: their stores issue spread-out and early, so the
# post-matmul store tail (and the straggler DMA engine's backlog) is short.
GROUPS = [(0, 0, 4), (0, 4, 5), (0, 9, 4)]
for _b in range(1, BPC - 2):
    GROUPS += [(_b, 0, 7), (_b, 7, 6)]
GROUPS += [
    (BPC - 2, 0, 7),
    (BPC - 1, 0, 7),
    (BPC - 2, 7, 6),
    (BPC - 1, 7, 6),
]
NG = len(GROUPS)                 # 17
NJW = 13                         # groups 0..NJW-1 get whole-image stores
N_STORE = (BPC - 2) + (NG - NJW) + 1  # 6 whole-image + 4 tail (last in halves)
# batch-0 row chunks: d1 = [w | rows 0-8], d2 = rows 9-26
B0C1 = 9 * ROWE
WARMUP = 8                       # PE clock-ramp matmuls on garbage SBUF

_cache = {}


def _build_bass():
    """Raw-bass build (no Tile): this toolchain's walrus codegen allows only
    ONE sync-wait per instruction, so all cross-engine sync is explicit
    single-sem waits; ordering beyond that rides on hardware transitivity.
    """
    import concourse.bass as bass
    import concourse.mybir as mybir

    f32 = mybir.dt.float32
    f16 = mybir.dt.float16

    nc = bass.Bass()
    # h = [w row | batch-0 image row] per partition — one dense descriptor
    # covers the whole first-matmul gate.
    h_d = nc.declare_dram_parameter("h", [128, WCOLS + RC], f16, isOutput=False)
    x_d = nc.declare_dram_parameter(
        "x", [BPC - 1, 128, RC], f16, isOutput=False
    )
    b_d = nc.declare_dram_parameter("b", [128, 1], f32, isOutput=False)
    o_d = nc.declare_dram_parameter("out", [BPC, 128, NOUT], f16, isOutput=True)

    D1 = WCOLS + B0C1            # w + b0 rows 0-8 in descriptor 1

    from contextlib import ExitStack

    with ExitStack() as es:
        ht = es.enter_context(nc.sbuf_tensor([128, WCOLS + RC], f16))
        bt = es.enter_context(nc.sbuf_tensor([128, 1], f32))
        gt = es.enter_context(nc.sbuf_tensor([128, BPC - 1, RC], f16))
        ot = es.enter_context(nc.sbuf_tensor([128, BPC * NOUT], f16))
        junk_out = es.enter_context(nc.sbuf_tensor([128, 16], f16))
        ps = es.enter_context(nc.psum_tensor([128, 8, 512], f32))
        d_sems = [es.enter_context(nc.semaphore(f"d{i}s")) for i in range(2)]
        bias_sem = es.enter_context(nc.semaphore("bias_sem"))
        g_sems = []
        for b in range(1, BPC):
            if b < NSPLIT:
                g_sems.append(
                    (
                        es.enter_context(nc.semaphore(f"g{b}hs")),
                        es.enter_context(nc.semaphore(f"g{b}ts")),
                    )
                )
            else:
                s = es.enter_context(nc.semaphore(f"g{b}s"))
                g_sems.append((s, s))
        warm_sem = es.enter_context(nc.semaphore("warm_sem"))
        vact_sem = es.enter_context(nc.semaphore("vact_sem"))
        pe_sem = es.enter_context(nc.semaphore("pe_sem"))
        act_sem = es.enter_context(nc.semaphore("act_sem"))
        out_sem = es.enter_context(nc.semaphore("out_sem"))
        block = es.enter_context(nc.Block())
        wtr = ht[:, :WCOLS].rearrange("p (k m) -> p k m", k=9)

        def image(b):
            base = ht[:, WCOLS:] if b == 0 else gt[:, b - 1, :]
            return base.rearrange("p (r c w) -> p r c w", r=RDIM, c=RDIM)

        # All inputs on the sync HW queue in need order.
        @block.sync
        def _(sync):
            sync.dma_start(ht[:, :D1], h_d[:, :D1]).then_inc(d_sems[0], 16)
            sync.dma_start(ht[:, D1:], h_d[:, D1:]).then_inc(d_sems[1], 16)
            for b in range(1, BPC):
                if b < NSPLIT:
                    sync.dma_start(
                        gt[:, b - 1, :HCUT], x_d[b - 1][:, :HCUT]
                    ).then_inc(g_sems[b - 1][0], 16)
                    if b == 1:
                        # bias is only needed by the first ACT (~1.5us
                        # after b1's head) — keep it off the critical
                        # prefix of the stream.
                        sync.dma_start(bt[:, :], b_d[:, :]).then_inc(
                            bias_sem, 16
                        )
                    sync.dma_start(
                        gt[:, b - 1, HCUT:], x_d[b - 1][:, HCUT:]
                    ).then_inc(g_sems[b - 1][1], 16)
                else:
                    sync.dma_start(
                        gt[:, b - 1, :], x_d[b - 1][:, :]
                    ).then_inc(g_sems[b - 1][0], 16)
            # Tail-group store 14 and the second half of the final group's
            # store ride the (drained by then) sync queue, in parallel
            # with scalar's.
            for j in range(NJW + 1, NG - 1, 2):
                b, X0, nX = GROUPS[j]
                sync.wait_ge(act_sem, j + 1)
                sync.dma_start(
                    o_d[b][:, X0 * XGRP : (X0 + nX) * XGRP],
                    ot[:, b * NOUT + X0 * XGRP : b * NOUT + (X0 + nX) * XGRP],
                ).then_inc(out_sem, 16)
            b, X0, nX = GROUPS[NG - 1]
            h0 = (X0 + 3) * XGRP
            h1 = (X0 + nX) * XGRP
            sync.wait_ge(act_sem, NG - 1)
            sync.dma_start(
                o_d[b][:, h0:h1], ot[:, b * NOUT + h0 : b * NOUT + h1]
            ).then_inc(out_sem, 16)
            sync.wait_ge(out_sem, 16 * N_STORE)

        @block.gpsimd
        def _(gpsimd):
            # Dummy read to warm the gpsimd queue's DGE context — the first
            # descriptor on a cold queue pays ~2us before bytes move.
            gpsimd.dma_start(junk_out[:, :2], h_d[:, :2]).then_inc(
                warm_sem, 16
            )
            # Per-group stores for tail groups 13/15, in parallel with
            # sync's 14 + final-half and scalar's ACTs. Group NG-2's bias
            # add runs on vector (vact_sem), overlapping scalar's final
            # ACT.
            for j in range(NJW, NG - 1, 2):
                b, X0, nX = GROUPS[j]
                if j == NG - 2:
                    gpsimd.wait_ge(vact_sem, 1)
                else:
                    gpsimd.wait_ge(act_sem, j + 1)
                gpsimd.dma_start(
                    o_d[b][:, X0 * XGRP : (X0 + nX) * XGRP],
                    ot[:, b * NOUT + X0 * XGRP : b * NOUT + (X0 + nX) * XGRP],
                ).then_inc(out_sem, 16)

        @block.tensor
        def _(tensor):
            # Warm the PE HAM clock gate (cold = half duty) on garbage SBUF
            # (ot staging tile: nothing writes it until the first ACT, well
            # after the warm-ups). Results land in PSUM bank 7 and are
            # overwritten by group 7's start=True accumulation.
            for i in range(WARMUP):
                tensor.matmul(
                    ps[:, 7, :512], ot[:, :128], ot[:, :512], start=True,
                    stop=True,
                )
            for j, (b, X0, nX) in enumerate(GROUPS):
                if j < 3:
                    tensor.wait_ge(d_sems[min(j, 1)], 16)
                else:
                    tensor.wait_ge(g_sems[b - 1][0 if X0 == 0 else 1], 16)
                if j >= 8:
                    # PSUM bank j%8 is free once ACT drained group j-8
                    tensor.wait_ge(act_sem, j - 7)
                gr = image(b)
                for kk in range(9):
                    kx, ky = divmod(kk, 3)
                    rhs = gr[
                        :,
                        2 * X0 + kx : 2 * X0 + kx + 2 * nX - 1 : 2,
                        ky : ky + 2 * WOUT - 1 : 2,
                        :,
                    ]
                    mm = tensor.matmul(
                        ps[:, j % 8, : nX * XGRP],
                        wtr[:, kk, :],
                        rhs,
                        start=(kk == 0),
                        stop=(kk == 8),
                    )
                mm.then_inc(pe_sem, 1)

        @block.scalar
        def _(scalar):
            # Hoist ACT_TABLE_LOAD: walrus emits it before the first ACT.
            # Reads garbage ht with a const bias — no data dependency.
            scalar.activation(
                junk_out[:, :],
                ht[:, :16],
                mybir.ActivationFunctionType.Identity,
                bias=0.0,
            )
            scalar.wait_ge(bias_sem, 16)
            for j, (b, X0, nX) in enumerate(GROUPS):
                if j == NG - 2:
                    # group NG-2's bias add runs on vector instead, in
                    # parallel with the final group's ACT here
                    continue
                off = b * NOUT + X0 * XGRP
                n = nX * XGRP
                scalar.wait_ge(pe_sem, j + 1)
                scalar.activation(
                    ot[:, off : off + n],
                    ps[:, j % 8, :n],
                    mybir.ActivationFunctionType.Identity,
                    bias=bt[:, :],
                ).then_inc(act_sem, 1)
                # One whole-image store per batch 0..4 right after that
                # batch's last ACT, on the scalar HW queue (starved while
                # the sync queue drains inputs — harmless). Self-wait: ACT
                # write-back posted before the DMA doorbell.
                if j < NJW and (j == NJW - 1 or GROUPS[j + 1][0] != b):
                    scalar.wait_ge(act_sem, j + 1)
                    scalar.dma_start(
                        o_d[b][:, :], ot[:, b * NOUT : (b + 1) * NOUT]
                    ).then_inc(out_sem, 16)
            # First half of the final group's store issued by scalar
            # itself; sync issues the second half, gpsimd the other tail
            # groups, all in parallel.
            b, X0, nX = GROUPS[NG - 1]
            h0 = X0 * XGRP
            h1 = (X0 + 3) * XGRP
            scalar.wait_ge(act_sem, NG - 1)
            scalar.dma_start(
                o_d[b][:, h0:h1], ot[:, b * NOUT + h0 : b * NOUT + h1]
            ).then_inc(out_sem, 16)

        @block.vector
        def _(vector):
            # Bias add for group NG-2 on the (otherwise idle) vector
            # engine, overlapping scalar's final-group ACT.
            b, X0, nX = GROUPS[NG - 2]
            off = b * NOUT + X0 * XGRP
            n = nX * XGRP
            vector.wait_ge(bias_sem, 16)
            vector.wait_ge(pe_sem, NG - 1)
            vector.tensor_scalar_add(
                ot[:, off : off + n], ps[:, (NG - 2) % 8, :n], bt[:, :]
            ).then_inc(vact_sem, 1)

    return nc


def _prep_inputs(x, W, bias):
    # x: (B, CIN, 28, 28, 16) -> xp[b, cin*4+v, (r*27+c)*4+w] = x[b,cin,r,c,4v+w]
    # for r,c in [0,27) — row/col 27 are never read by the stride-2 window.
    # fp16: PE runs fp32 matmuls as LOW_HIGH double passes; fp16 is single-pass
    # with fast-weight-load, and halves the dominant HBM traffic. Max rel err
    # ~5e-4 at this contraction depth (fp32 PSUM accumulation).
    xp = np.ascontiguousarray(
        x.reshape(B, CIN, WIN, WIN, H, H)[:, :, :RDIM, :RDIM]
        .transpose(0, 1, 4, 2, 3, 5)
    ).reshape(B, CIN * H, RC).astype(np.float16)
    # W: (1, 288, 32, 1, 1, 4, 4); p = cin*9 + kx*3 + ky
    # wt_sb[cin*4+v, kk*128 + o*4+u] = Wm[cin*9+kk, o, u, v]
    Wm = np.asarray(W, dtype=np.float32).reshape(CIN, KK * KK, COUT, H, H)
    wt_sb = np.ascontiguousarray(
        Wm.transpose(0, 4, 1, 2, 3)  # cin, v, kk, o, u
    ).reshape(128, WCOLS).astype(np.float16)
    bias_v = np.ascontiguousarray(
        np.repeat(np.asarray(bias, dtype=np.float32).reshape(COUT), H)
    ).reshape(128, 1)
    return xp, wt_sb, bias_v


def _shard(xp, wt_sb, core):
    # per-core inputs: h = [w | batch0 image] fp16, x = batches 1..7
    xs = xp[core * BPC : (core + 1) * BPC]
    h = np.ascontiguousarray(np.concatenate([wt_sb, xs[0]], axis=1))
    return h, np.ascontiguousarray(xs[1:])


def _unprep_output(full):
    # full: (B, 128, NOUT) fp16 with partition o*4+u, free (X, Y, w)
    out = (
        full.astype(np.float32)
        .reshape(B, COUT, H, WOUT, WOUT, H)
        .transpose(0, 1, 3, 4, 2, 5)
        .reshape(B, COUT, WOUT, WOUT, HH)
    )
    return np.ascontiguousarray(out)


def run_device(in_maps, trace=False, tmpdir=None):
    from concourse.bass_utils import run_bass_kernel_spmd

    if "nc" not in _cache:
        _cache["nc"] = _build_bass()
    return run_bass_kernel_spmd(
        _cache["nc"], in_maps, list(range(NCORES)), trace=trace, tmpdir=tmpdir
    )


def _in_maps(x, W, bias):
    xp, wt_sb, bias_v = _prep_inputs(x, W, bias)
    maps = []
    for i in range(NCORES):
        h, xs = _shard(xp, wt_sb, i)
        maps.append({"h": h, "x": xs, "b": bias_v})
    return maps


def kernel(x, W, bias):
    x = np.asarray(x, dtype=np.float32)
    res = run_device(_in_maps(x, W, bias), trace=False)
    full = np.concatenate(
        [res.results[i]["out"] for i in range(NCORES)], axis=0
    )
    return _unprep_output(full)
